# revision 43
# baseline (speedup 1.0000x reference)
"""Trainium2 Bass kernel: 3x3 conv (N=16, C_in=16, C_out=64, H=W=256, pad=1).

Strategy (8 NeuronCores, data-parallel over batch N -> 2 images/core):
  - Host pads x to [2,16,258,258] (zero ring) and converts to bf16; W goes
    to a [96,384] bf16 block-diagonal lhsT; outputs come back fp16 and are
    upcast to fp32 on the host.  Halving both DMA directions matters: the
    DMA engines are the binding resource.
  - Per 64-row superstep: slab [96, 32*258] holds partitions
    (strip, kh, ci); each (strip, kh) group is loaded DIRECTLY from HBM at
    a kh-shifted row offset (one 4-level-src DMA per superstep), so no
    SBUF->SBUF shift copies and no Activation-engine copies are needed.
  - One matmul per kw tap (3 total, PSUM-accumulated) with a [96,128]
    block-diagonal bf16 weight tile computes both strips' 64 output
    channels for 512 pixels (2 rows x 256); kw shifts are free-dim offsets
    into the 258-pitch row slots.
  - PSUM -> SBUF evacuation alternates DVE / Activation (fp32 -> fp16
    downcast); one [128, 2048] store per 4-bank group covers both strips
    via a 3-level DRAM AP.  A single early dummy matmul warms the PE
    p-state ramp; the first slab loads in pieces so compute starts ~4us in;
    the last group stores in small pieces to shorten the drain.
"""

import sys

if "/opt/trn_rl_repo" not in sys.path:
    sys.path.insert(0, "/opt/trn_rl_repo")

import ml_dtypes
import numpy as np

import concourse.bacc as bacc
import concourse.bass as bass
import concourse.mybir as mybir
import concourse.tile as tile
from concourse.bass_utils import run_bass_kernel_spmd

N_FULL, CI, CO, H, W_SP = 16, 16, 64, 256, 256
NCORES = 8
NB = N_FULL // NCORES          # batches per core
HP, WP = H + 2, W_SP + 2       # padded image dims
SLOT = WP                      # 258: one row-slot in the slab (z x0..x255 z)
RSTRIP = 32                    # output rows per strip
NSS = H // (2 * RSTRIP)        # supersteps per image (4)
GF = RSTRIP * SLOT             # free elems per (strip,kh) group: 8256
BF16 = mybir.dt.bfloat16
F16 = mybir.dt.float16
F32 = mybir.dt.float32
WARMUP_MM = 1                  # PE pre-warm matmuls during the DMA fill

_CACHE = {}


def _build(reps: int = 1):
    nc = bacc.Bacc("TRN2", target_bir_lowering=False, debug=False)
    x_d = nc.dram_tensor("xp", [NB, CI, HP, WP], BF16, kind="ExternalInput").ap()
    w_d = nc.dram_tensor("wts", [96, 384], BF16, kind="ExternalInput").ap()
    o_d = nc.dram_tensor("out", [NB, CO, H, W_SP], F16, kind="ExternalOutput").ap()

    xe_n = CI * HP * WP        # x_pad element strides
    xe_c = HP * WP
    xe_h = WP
    oe_n = CO * H * W_SP
    oe_c = H * W_SP

    with tile.TileContext(nc) as tc:
        with (
            tc.tile_pool(name="wp", bufs=1) as wpool,
            tc.tile_pool(name="slab", bufs=3) as slabpool,
            tc.tile_pool(name="evac", bufs=6) as evacpool,
            tc.tile_pool(name="ps", bufs=8, space="PSUM") as pspool,
        ):
            wsb = wpool.tile([96, 384], BF16)

            def build_slab(n, t, pieces=(RSTRIP,), after_first=None):
                # partition p = strip*48 + kh*16 + ci; slot u of group
                # (strip,kh) holds xpad row (64t + 32*strip + kh + u), so a
                # bank's rhs slice IS the kh-shifted window (no copies).
                # Multiple row-pieces (prologue only) let the first banks'
                # matmuls start before the full slab lands; strip B goes
                # through the Activation HWDGE queue to parallelize
                # descriptor generation with Pool's SWDGE.
                slab = slabpool.tile([96, GF], BF16, tag="slab")
                r0 = 0
                for pi, rows in enumerate(pieces):
                    for strip in range(2):
                        src = bass.AP(
                            x_d.tensor,
                            n * xe_n + ((2 * t + strip) * RSTRIP + r0) * xe_h,
                            [[xe_h, 3], [xe_c, CI], [1, rows * SLOT]],
                        )
                        eng = nc.gpsimd if (len(pieces) == 1 or strip == 0) else nc.sync
                        eng.dma_start(
                            slab[
                                48 * strip : 48 * (strip + 1),
                                r0 * SLOT : (r0 + rows) * SLOT,
                            ],
                            src,
                        )
                    r0 += rows
                    if pi == 0 and after_first is not None:
                        after_first()
                return slab

            def compute(n, t, slab, fine_tail=False):
                su = slab[:].rearrange("p (u e) -> p u e", u=RSTRIP)
                for grp in range(4):
                    evac = evacpool.tile([128, 4 * 512], F16, tag="evac")
                    for bb in range(4):
                        b = 4 * grp + bb
                        ps = pspool.tile([128, 512], F32, tag="ps")
                        for kw in range(3):
                            rhs = su[:, 2 * b : 2 * b + 2, kw : kw + 256]
                            nc.tensor.matmul(
                                ps[:],
                                wsb[:, kw * 128 : (kw + 1) * 128],
                                rhs,
                                start=(kw == 0),
                                stop=(kw == 2),
                            )
                        dst = evac[:, bb * 512 : (bb + 1) * 512]
                        if bb % 2 == 0:
                            nc.vector.tensor_copy(dst, ps[:])
                        else:
                            nc.scalar.copy(dst, ps[:])
                    # one store covers both strips via a 3-level DRAM AP;
                    # finer pieces on the last group so the drain only waits
                    # on the final banks
                    if fine_tail and grp == 3:
                        pieces_ = [(0, 4), (4, 2), (6, 2)]   # (row offset, rows)
                    else:
                        pieces_ = [(0, 8)]
                    for c0, prows in pieces_:
                        dstrow = 2 * RSTRIP * t + 8 * grp + c0
                        nc.sync.dma_start(
                            bass.AP(
                                o_d.tensor,
                                n * oe_n + dstrow * W_SP,
                                [[RSTRIP * W_SP, 2], [oe_c, CO], [1, prows * W_SP]],
                            ),
                            evac[:, c0 * 256 : c0 * 256 + prows * 256],
                        )

            # PE pre-warm: the cost model ramps the PE clock only after a few
            # microseconds of continuous execution, so burn dummy matmuls
            # during the initial slab-load fill to hit full speed by the time
            # real matmuls start.
            warm = wpool.tile([128, 512], BF16)
            nc.vector.memset(warm[:], 0.0)
            wps = pspool.tile([128, 512], F32, tag="ps")
            for _ in range(WARMUP_MM):
                nc.tensor.matmul(wps[:], warm[:, 0:128], warm[:], start=True, stop=True)

            # software pipeline with two-superstep lookahead on slab loads
            LOOK = 2
            steps = [(n, t) for _ in range(reps) for n in range(NB) for t in range(NSS)]
            slabs = {}
            nc.sync.dma_start(wsb[:], w_d[:, :])
            slabs[steps[0]] = build_slab(*steps[0], pieces=(4, 4, 8, 16))
            for k in range(1, min(LOOK, len(steps))):
                slabs[steps[k]] = build_slab(*steps[k])
            for i, (n, t) in enumerate(steps):
                if i + LOOK < len(steps):
                    slabs[steps[i + LOOK]] = build_slab(*steps[i + LOOK])
                compute(n, t, slabs.pop((n, t)), fine_tail=(i == len(steps) - 1))

    nc.compile()
    return nc


def _prep_weights(W: np.ndarray) -> np.ndarray:
    # lhsT[strip*48 + kh*16 + ci, kw*128 + strip*64 + co] = W[co, ci, kh, kw]
    wts = np.zeros((96, 384), dtype=np.float32)
    blk = np.ascontiguousarray(W.astype(np.float32).transpose(2, 1, 0, 3))  # [kh,ci,co,kw]
    for strip in range(2):
        for kw in range(3):
            wts[strip * 48 : (strip + 1) * 48,
                kw * 128 + strip * 64 : kw * 128 + strip * 64 + 64] = (
                blk[:, :, :, kw].reshape(48, 64)
            )
    return wts.astype(ml_dtypes.bfloat16)


def kernel(x: np.ndarray, W: np.ndarray) -> np.ndarray:
    assert x.shape == (N_FULL, CI, H, W_SP) and W.shape == (CO, CI, 3, 3)
    # BASS_TRACE without the axon NTFF hook module would crash the run path;
    # disable tracing only when the hook is genuinely unavailable.
    try:
        import antenv.axon_hooks  # noqa: F401
    except Exception:
        import os

        os.environ.setdefault("BASS_NEVER_TRACE", "1")
    if "nc" not in _CACHE:
        _CACHE["nc"] = _build()
    nc = _CACHE["nc"]

    wts = _prep_weights(np.asarray(W))
    xs = np.asarray(x, dtype=np.float32).reshape(NCORES, NB, CI, H, W_SP)
    in_maps = []
    for i in range(NCORES):
        xp = np.zeros((NB, CI, HP, WP), dtype=ml_dtypes.bfloat16)
        xp[:, :, 1 : H + 1, 1 : W_SP + 1] = xs[i].astype(ml_dtypes.bfloat16)
        in_maps.append({"xp": xp, "wts": wts})

    res = run_bass_kernel_spmd(nc, in_maps, list(range(NCORES)))
    out = np.concatenate(
        [np.asarray(res.results[i]["out"]) for i in range(NCORES)], axis=0
    )
    return out.astype(np.float32)


# revision 46
# speedup vs baseline: 1.0016x; 1.0016x over previous
"""Trainium2 Bass kernel: 3x3 conv (N=16, C_in=16, C_out=64, H=W=256, pad=1).

Strategy (8 NeuronCores, data-parallel over batch N -> 2 images/core):
  - Host pads x to [2,16,258,258] (zero ring) and converts to bf16; W goes
    to a [96,384] bf16 block-diagonal lhsT; outputs come back fp16 and are
    upcast to fp32 on the host.  Halving both DMA directions matters: the
    DMA engines are the binding resource.
  - Per 64-row superstep: slab [96, 32*258] holds partitions
    (strip, kh, ci); each (strip, kh) group is loaded DIRECTLY from HBM at
    a kh-shifted row offset (one 4-level-src DMA per superstep), so no
    SBUF->SBUF shift copies and no Activation-engine copies are needed.
  - One matmul per kw tap (3 total, PSUM-accumulated) with a [96,128]
    block-diagonal bf16 weight tile computes both strips' 64 output
    channels for 512 pixels (2 rows x 256); kw shifts are free-dim offsets
    into the 258-pitch row slots.
  - PSUM -> SBUF evacuation alternates DVE / Activation (fp32 -> fp16
    downcast); one [128, 2048] store per 4-bank group covers both strips
    via a 3-level DRAM AP.  A single early dummy matmul warms the PE
    p-state ramp; the first slab loads in pieces so compute starts ~4us in;
    the last group stores in small pieces to shorten the drain.
"""

import sys

if "/opt/trn_rl_repo" not in sys.path:
    sys.path.insert(0, "/opt/trn_rl_repo")

import ml_dtypes
import numpy as np

import concourse.bacc as bacc
import concourse.bass as bass
import concourse.mybir as mybir
import concourse.tile as tile
from concourse.bass_utils import run_bass_kernel_spmd

N_FULL, CI, CO, H, W_SP = 16, 16, 64, 256, 256
NCORES = 8
NB = N_FULL // NCORES          # batches per core
HP, WP = H + 2, W_SP + 2       # padded image dims
SLOT = WP                      # 258: one row-slot in the slab (z x0..x255 z)
RSTRIP = 32                    # output rows per strip
NSS = H // (2 * RSTRIP)        # supersteps per image (4)
GF = RSTRIP * SLOT             # free elems per (strip,kh) group: 8256
BF16 = mybir.dt.bfloat16
F16 = mybir.dt.float16
F32 = mybir.dt.float32
WARMUP_MM = 1                  # PE pre-warm matmuls during the DMA fill

_CACHE = {}


def _build(reps: int = 1):
    nc = bacc.Bacc("TRN2", target_bir_lowering=False, debug=False)
    x_d = nc.dram_tensor("xp", [NB, CI, HP, WP], BF16, kind="ExternalInput").ap()
    w_d = nc.dram_tensor("wts", [96, 384], BF16, kind="ExternalInput").ap()
    o_d = nc.dram_tensor("out", [NB, CO, H, W_SP], F16, kind="ExternalOutput").ap()

    xe_n = CI * HP * WP        # x_pad element strides
    xe_c = HP * WP
    xe_h = WP
    oe_n = CO * H * W_SP
    oe_c = H * W_SP

    with tile.TileContext(nc) as tc:
        with (
            tc.tile_pool(name="wp", bufs=1) as wpool,
            tc.tile_pool(name="slab", bufs=3) as slabpool,
            tc.tile_pool(name="evac", bufs=6) as evacpool,
            tc.tile_pool(name="ps", bufs=8, space="PSUM") as pspool,
        ):
            wsb = wpool.tile([96, 384], BF16)

            def build_slab(n, t, pieces=(RSTRIP,), after_first=None):
                # partition p = strip*48 + kh*16 + ci; slot u of group
                # (strip,kh) holds xpad row (64t + 32*strip + kh + u), so a
                # bank's rhs slice IS the kh-shifted window (no copies).
                # Multiple row-pieces (prologue only) let the first banks'
                # matmuls start before the full slab lands; strip B then
                # goes through the SP HWDGE queue to parallelize descriptor
                # generation with Pool's SWDGE (strip A).
                slab = slabpool.tile([96, GF], BF16, tag="slab")
                r0 = 0
                for pi, rows in enumerate(pieces):
                    for strip in range(2):
                        src = bass.AP(
                            x_d.tensor,
                            n * xe_n + ((2 * t + strip) * RSTRIP + r0) * xe_h,
                            [[xe_h, 3], [xe_c, CI], [1, rows * SLOT]],
                        )
                        eng = nc.gpsimd if (len(pieces) == 1 or strip == 0) else nc.sync
                        eng.dma_start(
                            slab[
                                48 * strip : 48 * (strip + 1),
                                r0 * SLOT : (r0 + rows) * SLOT,
                            ],
                            src,
                        )
                    r0 += rows
                    if pi == 0 and after_first is not None:
                        after_first()
                return slab

            def compute(n, t, slab, fine_tail=False):
                su = slab[:].rearrange("p (u e) -> p u e", u=RSTRIP)
                for grp in range(4):
                    evac = evacpool.tile([128, 4 * 512], F16, tag="evac")
                    for bb in range(4):
                        b = 4 * grp + bb
                        ps = pspool.tile([128, 512], F32, tag="ps")
                        for kw in range(3):
                            rhs = su[:, 2 * b : 2 * b + 2, kw : kw + 256]
                            nc.tensor.matmul(
                                ps[:],
                                wsb[:, kw * 128 : (kw + 1) * 128],
                                rhs,
                                start=(kw == 0),
                                stop=(kw == 2),
                            )
                        dst = evac[:, bb * 512 : (bb + 1) * 512]
                        if bb % 2 == 0:
                            nc.vector.tensor_copy(dst, ps[:])
                        else:
                            nc.scalar.copy(dst, ps[:])
                    # one store covers both strips via a 3-level DRAM AP;
                    # finer pieces on the last group so the drain only waits
                    # on the final banks
                    if fine_tail and grp == 3:
                        pieces_ = [(0, 4), (4, 2), (6, 2)]   # (row offset, rows)
                    else:
                        pieces_ = [(0, 8)]
                    for pk, (c0, prows) in enumerate(pieces_):
                        dstrow = 2 * RSTRIP * t + 8 * grp + c0
                        seng = nc.scalar if (len(pieces_) > 1 and pk == 1) else nc.sync
                        seng.dma_start(
                            bass.AP(
                                o_d.tensor,
                                n * oe_n + dstrow * W_SP,
                                [[RSTRIP * W_SP, 2], [oe_c, CO], [1, prows * W_SP]],
                            ),
                            evac[:, c0 * 256 : c0 * 256 + prows * 256],
                        )

            # PE pre-warm: the cost model ramps the PE clock only after a few
            # microseconds of continuous execution, so burn dummy matmuls
            # during the initial slab-load fill to hit full speed by the time
            # real matmuls start.
            warm = wpool.tile([128, 512], BF16)
            nc.vector.memset(warm[:], 0.0)
            wps = pspool.tile([128, 512], F32, tag="ps")
            for _ in range(WARMUP_MM):
                nc.tensor.matmul(wps[:], warm[:, 0:128], warm[:], start=True, stop=True)

            # software pipeline with two-superstep lookahead on slab loads
            LOOK = 2
            steps = [(n, t) for _ in range(reps) for n in range(NB) for t in range(NSS)]
            slabs = {}
            nc.sync.dma_start(wsb[:], w_d[:, :])
            slabs[steps[0]] = build_slab(*steps[0], pieces=(4, 4, 8, 16))
            for k in range(1, min(LOOK, len(steps))):
                slabs[steps[k]] = build_slab(*steps[k])
            for i, (n, t) in enumerate(steps):
                if i + LOOK < len(steps):
                    slabs[steps[i + LOOK]] = build_slab(*steps[i + LOOK])
                compute(n, t, slabs.pop((n, t)), fine_tail=(i == len(steps) - 1))

    nc.compile()
    return nc


def _prep_weights(W: np.ndarray) -> np.ndarray:
    # lhsT[strip*48 + kh*16 + ci, kw*128 + strip*64 + co] = W[co, ci, kh, kw]
    wts = np.zeros((96, 384), dtype=np.float32)
    blk = np.ascontiguousarray(W.astype(np.float32).transpose(2, 1, 0, 3))  # [kh,ci,co,kw]
    for strip in range(2):
        for kw in range(3):
            wts[strip * 48 : (strip + 1) * 48,
                kw * 128 + strip * 64 : kw * 128 + strip * 64 + 64] = (
                blk[:, :, :, kw].reshape(48, 64)
            )
    return wts.astype(ml_dtypes.bfloat16)


def kernel(x: np.ndarray, W: np.ndarray) -> np.ndarray:
    assert x.shape == (N_FULL, CI, H, W_SP) and W.shape == (CO, CI, 3, 3)
    # BASS_TRACE without the axon NTFF hook module would crash the run path;
    # disable tracing only when the hook is genuinely unavailable.
    try:
        import antenv.axon_hooks  # noqa: F401
    except Exception:
        import os

        os.environ.setdefault("BASS_NEVER_TRACE", "1")
    if "nc" not in _CACHE:
        _CACHE["nc"] = _build()
    nc = _CACHE["nc"]

    wts = _prep_weights(np.asarray(W))
    xs = np.asarray(x, dtype=np.float32).reshape(NCORES, NB, CI, H, W_SP)
    in_maps = []
    for i in range(NCORES):
        xp = np.zeros((NB, CI, HP, WP), dtype=ml_dtypes.bfloat16)
        xp[:, :, 1 : H + 1, 1 : W_SP + 1] = xs[i].astype(ml_dtypes.bfloat16)
        in_maps.append({"xp": xp, "wts": wts})

    res = run_bass_kernel_spmd(nc, in_maps, list(range(NCORES)))
    out = np.concatenate(
        [np.asarray(res.results[i]["out"]) for i in range(NCORES)], axis=0
    )
    return out.astype(np.float32)
